# revision 1
# baseline (speedup 1.0000x reference)
"""Trainium2 Bass kernel for the Koopman operator nn.Module.

Per-channel tiny MLPs (4 real channels, 6 complex-conjugate pairs, H=64,
2 hidden layers) over 65536 flattened batch elements, then a block-diagonal
Koopman update.  Pure data parallel over 8 NeuronCores (8192 elements each).

Design notes (v8):
  - elements on the free dim, hidden units on partitions; channels in
    block-diagonal PAIRS: hidden matmuls are [128,128]x[128,512] f32r.
  - all MLP biases are zero (spec fill=zeros; asserted on host), so the
    relu passes are pure max(x,0).
  - the 15 relu passes alternate between DVE and ACT per pair so each
    pair's relu hides under the other four pairs' matmuls; GPSIMD (Pool)
    cannot touch PSUM on real HW, so it gets all the SBUF-side work
    (x-prep, polynomial trig, rotation combines).
  - emission is software-pipelined: tile t+1's prelude and tile t-1's
    epilogue are interleaved between tile t's MLP stages so every
    engine FIFO always has independent work queued.
  - final layer packs all 16 outputs (lambda 0-3 | mu 4-9 | omega 10-15)
    into a [16,512] PSUM accumulator that shares the hidden-ps ring slot
    rotation; the elem-major T tile shares the xT ring. 6 hidden ring
    slots + xT + stk rings = 8 PSUM banks.
  - ACT keeps ONE table set resident (exp_and_others: relu+exp+identity);
    sin/cos are degree 5/4 polynomials (|omega| < 0.5 with big margin),
    so there is no per-tile activation-table reload.
  - weights ride in 3 consolidated DMAs; per-pair weights are column
    slices of the big SBUF tiles.
"""

import numpy as np

NR, NCC, L, H = 4, 6, 2, 64
B, S, C = 32, 2048, 16
NCORES = 8
F_CORE = B * S // NCORES        # 8192 elements per core
TILE = 512                      # elements per compute tile
GROUPS = TILE // 128            # 4
NT = F_CORE // TILE             # 16

_cached_nc = None

# relu engine per [pair][stage]; DVE and ACT alternate within a stage
# (8 on DVE, 7 on ACT: real-HW ACT passes cost ~720ns vs DVE ~658ns)
RELU_ASSIGN = [
    ["dve", "act", "dve"],
    ["act", "dve", "act"],
    ["dve", "act", "dve"],
    ["act", "dve", "act"],
    ["dve", "act", "dve"],
]


def _build():
    import concourse.tile as tile
    from concourse import bacc, mybir
    from concourse.masks import make_identity

    f32 = mybir.dt.float32
    f32r = mybir.dt.float32r
    RELU = mybir.ActivationFunctionType.Relu
    EXP = mybir.ActivationFunctionType.Exp
    IDENT = mybir.ActivationFunctionType.Identity
    ADD = mybir.AluOpType.add
    SUB = mybir.AluOpType.subtract
    MULT = mybir.AluOpType.mult

    nc = bacc.Bacc("TRN2", target_bir_lowering=False, debug=False,
                   num_devices=NCORES)

    def relu0(name, out, in_):
        if name == "act":
            nc.scalar.activation(out, in_, RELU)
        else:
            nc.vector.tensor_scalar_max(out, in_, 0.0)

    z = nc.dram_tensor("z", [F_CORE, C], f32, kind="ExternalInput").ap()
    w0p = nc.dram_tensor("w0p", [10, 5 * 128], f32r, kind="ExternalInput").ap()
    wmp = nc.dram_tensor("wmp", [128, L * 5 * 128], f32r,
                         kind="ExternalInput").ap()
    wlp = nc.dram_tensor("wlp", [128, 5 * 16], f32r, kind="ExternalInput").ap()
    out = nc.dram_tensor("out", [F_CORE, C], f32, kind="ExternalOutput").ap()

    z_r = z.rearrange("(t g p) c -> t p g c", g=GROUPS, p=128)
    out_r = out.rearrange("(t g p) c -> t p g c", g=GROUPS, p=128)

    half = TILE // 2

    with tile.TileContext(nc) as tc:
        with (
            tc.tile_pool(name="singles", bufs=1) as singles,
            tc.tile_pool(name="io", bufs=6) as io,
            tc.tile_pool(name="acts", bufs=12) as acts,
            tc.tile_pool(name="epi", bufs=14) as epi,
            tc.tile_pool(name="pshid", bufs=6, space="PSUM") as pshid,
            tc.tile_pool(name="psxT", bufs=2, space="PSUM") as psxT,
        ):
            ident = singles.tile([128, 128], f32, tag="ident")
            make_identity(nc, ident)
            ident_r = singles.tile([128, 128], f32r, tag="ident_r")
            nc.vector.tensor_copy(ident_r, ident)

            # --- 3 consolidated weight DMAs; per-pair views are column
            # slices of the big SBUF tiles ---
            w0_all = singles.tile([10, 5 * 128], f32r, tag="w0_all")
            nc.sync.dma_start(out=w0_all, in_=w0p)
            wm_all = singles.tile([128, L * 5 * 128], f32r, tag="wm_all")
            nc.sync.dma_start(out=wm_all, in_=wmp)
            wl_all = singles.tile([128, 5 * 16], f32r, tag="wl_all")
            nc.sync.dma_start(out=wl_all, in_=wlp)

            w0_sb = [w0_all[:, j * 128:(j + 1) * 128] for j in range(5)]
            wm_sb = [[wm_all[:, (l * 5 + j) * 128:(l * 5 + j + 1) * 128]
                      for j in range(5)] for l in range(L)]
            wl_sb = [wl_all[:, j * 16:(j + 1) * 16] for j in range(5)]

            # GPSIMD (Pool) on real HW supports only plain tensor_tensor
            # (add/sub/mult) in SBUF, so the trig polynomial uses TT chains
            # against broadcast constant tiles.
            P = nc.gpsimd
            k24 = singles.tile([128, GROUPS, 6], f32, tag="k24")
            P.memset(k24, 1.0 / 24)
            kmh = singles.tile([128, GROUPS, 6], f32, tag="kmh")
            P.memset(kmh, -0.5)
            k120 = singles.tile([128, GROUPS, 6], f32, tag="k120")
            P.memset(k120, 1.0 / 120)
            km6 = singles.tile([128, GROUPS, 6], f32, tag="km6")
            P.memset(km6, -1.0 / 6)

            def emit_prelude(t):
                """DMA z, build x_nat, transpose to xT, evacuate to SBUF."""
                z_nat = io.tile([128, GROUPS, C], f32, name=f"z_nat_{t}",
                                tag="z_nat")
                nc.sync.dma_start(out=z_nat, in_=z_r[t])
                z1 = z_nat[:, :, 4:16:2]
                z2 = z_nat[:, :, 5:16:2]

                x_nat = io.tile([128, GROUPS, 10], f32r, name=f"x_nat_{t}",
                                tag="x_nat")
                nc.vector.tensor_copy(x_nat[:, :, 0:4], z_nat[:, :, 0:4])
                m1 = epi.tile([128, GROUPS, 6], f32, name=f"m1_{t}", tag="m1")
                P.tensor_tensor(m1, z1, z1, MULT)
                m2 = epi.tile([128, GROUPS, 6], f32, name=f"m2_{t}", tag="m2")
                P.tensor_tensor(m2, z2, z2, MULT)
                P.tensor_tensor(x_nat[:, :, 4:10], m1, m2, ADD)

                xT_fl = psxT.tile([128, TILE], f32r, name=f"xT_{t}", tag="xt")
                xT_ps = xT_fl[0:10, :]
                for g in range(GROUPS):
                    nc.tensor.transpose(
                        xT_ps[:, g * 128:(g + 1) * 128], x_nat[:, g, :],
                        ident_r)
                # whole-op evacuations (no halves): real-HW ACT has a large
                # per-op constant that makes split passes a net loss
                xT = acts.tile([10, TILE], f32r, name=f"xTs_{t}", tag="xT_sb")
                nc.vector.tensor_copy(xT, xT_ps)
                return z_nat, xT

            def emit_late(t, z_nat, stk_sb):
                """T transpose + evac, exp, polynomial trig, rotation."""
                z1 = z_nat[:, :, 4:16:2]
                z2 = z_nat[:, :, 5:16:2]

                T_fl = psxT.tile([128, TILE], f32r, name=f"T_{t}", tag="xt")
                for g in range(GROUPS):
                    nc.tensor.transpose(
                        T_fl[:, g * 16:(g + 1) * 16],
                        stk_sb[:, g * 128:(g + 1) * 128],
                        ident_r[0:16, 0:16])
                T_sb = epi.tile([128, GROUPS, 16], f32, name=f"Ts_{t}",
                                tag="T_sb")
                nc.vector.tensor_copy(T_sb, T_fl[:, 0:GROUPS * 16])

                lamT = T_sb[:, :, 0:4]
                muT = T_sb[:, :, 4:10]
                omT = T_sb[:, :, 10:16]

                e = epi.tile([128, GROUPS, 6], f32, name=f"e_{t}", tag="e")
                nc.scalar.activation(e, muT, EXP)

                # cos(om)*e ~ e + r*e       with r = (w2/24 - 1/2)*w2
                # sin(om)*e ~ (p*om + om)*e with p = (w2/120 - 1/6)*w2
                w2 = epi.tile([128, GROUPS, 6], f32, name=f"w2_{t}", tag="w2")
                P.tensor_tensor(w2, omT, omT, MULT)
                tc_ = epi.tile([128, GROUPS, 6], f32, name=f"tc_{t}", tag="tc")
                P.tensor_tensor(tc_, w2, k24, MULT)
                P.tensor_tensor(tc_, tc_, kmh, ADD)
                rc = epi.tile([128, GROUPS, 6], f32, name=f"rc_{t}", tag="rc")
                P.tensor_tensor(rc, tc_, w2, MULT)
                mc = epi.tile([128, GROUPS, 6], f32, name=f"mc_{t}", tag="mc")
                P.tensor_tensor(mc, rc, e, MULT)
                P.tensor_tensor(mc, mc, e, ADD)
                ts_ = epi.tile([128, GROUPS, 6], f32, name=f"ts_{t}", tag="ts")
                P.tensor_tensor(ts_, w2, k120, MULT)
                P.tensor_tensor(ts_, ts_, km6, ADD)
                rs = epi.tile([128, GROUPS, 6], f32, name=f"rs_{t}", tag="rs")
                P.tensor_tensor(rs, ts_, w2, MULT)
                sn = epi.tile([128, GROUPS, 6], f32, name=f"sn_{t}", tag="sn")
                P.tensor_tensor(sn, rs, omT, MULT)
                P.tensor_tensor(sn, sn, omT, ADD)
                ms = epi.tile([128, GROUPS, 6], f32, name=f"ms_{t}", tag="ms")
                P.tensor_tensor(ms, sn, e, MULT)

                # out_r = zr*lam; o1 = z1*mc + z2*ms; o2 = z2*mc - z1*ms
                o_nat = io.tile([128, GROUPS, C], f32, name=f"o_nat_{t}",
                                tag="o_nat")
                P.tensor_tensor(o_nat[:, :, 0:4], z_nat[:, :, 0:4], lamT,
                                MULT)
                t1 = epi.tile([128, GROUPS, 6], f32, name=f"t1_{t}", tag="t1")
                t2 = epi.tile([128, GROUPS, 6], f32, name=f"t2_{t}", tag="t2")
                P.tensor_tensor(t1, z1, mc, MULT)
                P.tensor_tensor(t2, z2, ms, MULT)
                P.tensor_tensor(o_nat[:, :, 4:16:2], t1, t2, ADD)
                t3 = epi.tile([128, GROUPS, 6], f32, name=f"t3_{t}", tag="t3")
                t4 = epi.tile([128, GROUPS, 6], f32, name=f"t4_{t}", tag="t4")
                P.tensor_tensor(t3, z2, mc, MULT)
                P.tensor_tensor(t4, z1, ms, MULT)
                P.tensor_tensor(o_nat[:, :, 5:16:2], t3, t4, SUB)

                nc.sync.dma_start(out=out_r[t], in_=o_nat)

            # --- software-pipelined main loop ---
            late_args = None      # tile t-1 epilogue inputs
            cur = emit_prelude(0)
            for t in range(NT):
                z_nat, xT = cur
                rhs = [xT] * 5
                pss = [None] * 5

                for s in range(3):
                    weights = w0_sb if s == 0 else wm_sb[s - 1]
                    for j in range(5):
                        pss[j] = pshid.tile([128, TILE], f32,
                                            name=f"ps_{t}_{s}_{j}", tag="ps")
                        nc.tensor.matmul(pss[j], weights[j], rhs[j],
                                         start=True, stop=True)
                    if s == 0 and t + 1 < NT:
                        # next tile's prelude: PE transposes + Pool x-prep
                        # fill the gap before this tile's stage-1 matmuls
                        cur = emit_prelude(t + 1)
                    if s == 1 and late_args is not None:
                        # previous tile's epilogue fills the stage-2 gap
                        emit_late(*late_args)
                        late_args = None
                    for j in range(5):
                        h = acts.tile([128, TILE], f32r,
                                      name=f"h_{t}_{s}_{j}", tag="h")
                        relu0(RELU_ASSIGN[j][s], h, pss[j])
                        rhs[j] = h

                # stk shares the pshid ring slot rotation
                stk_fl = pshid.tile([128, TILE], f32, name=f"stk_{t}",
                                    tag="ps")
                stk_ps = stk_fl[0:16, :]
                for j in range(5):
                    nc.tensor.matmul(stk_ps, wl_sb[j], rhs[j],
                                     start=(j == 0), stop=(j == 4))

                # evacuate stk to SBUF on ACT (biases all zero per the spec)
                stk_sb = acts.tile([16, TILE], f32r, name=f"stks_{t}",
                                   tag="stk_sb")
                nc.scalar.activation(stk_sb, stk_ps, IDENT)

                late_args = (t, z_nat, stk_sb)

            emit_late(*late_args)

    nc.compile()
    return nc


def _pack_weights(i):
    """Pack per-channel weights into block-diagonal pair form.

    All biases must be zero (guaranteed by the problem spec, fill=zeros);
    the kernel folds that assumption into pure-relu passes.
    """
    f32 = np.float32
    for k in ("b0_r", "bm_r", "bl_r", "b0_c", "bm_c", "bl_c"):
        assert not np.any(np.asarray(i[k])), f"nonzero bias {k}"
    W0_r = np.asarray(i["W0_r"], f32)
    Wm_r = np.asarray(i["Wm_r"], f32)
    Wl_r = np.asarray(i["Wl_r"], f32)
    W0_c = np.asarray(i["W0_c"], f32)
    Wm_c = np.asarray(i["Wm_c"], f32)
    Wl_c = np.asarray(i["Wl_c"], f32)

    w0p = np.zeros((5, 10, 128), f32)
    wmp = np.zeros((L, 5, 128, 128), f32)
    wlp = np.zeros((5, 128, 16), f32)
    for j in range(5):
        if j < 2:
            a, b = 2 * j, 2 * j + 1
            W0, Wm = W0_r, Wm_r
        else:
            a, b = 2 * (j - 2), 2 * (j - 2) + 1
            W0, Wm = W0_c, Wm_c
        r0 = 2 * j if j < 2 else 4 + 2 * (j - 2)
        w0p[j, r0, 0:64] = W0[a]
        w0p[j, r0 + 1, 64:128] = W0[b]
        for l in range(L):
            wmp[l, j, 0:64, 0:64] = Wm[l, a]
            wmp[l, j, 64:128, 64:128] = Wm[l, b]
        if j < 2:
            wlp[j, 0:64, 2 * j] = Wl_r[a][:, 0]
            wlp[j, 64:128, 2 * j + 1] = Wl_r[b][:, 0]
        else:
            jc = j - 2
            wlp[j, 0:64, 4 + 2 * jc] = Wl_c[a][:, 0]        # mu_a
            wlp[j, 64:128, 5 + 2 * jc] = Wl_c[b][:, 0]      # mu_b
            wlp[j, 0:64, 10 + 2 * jc] = Wl_c[a][:, 1]       # om_a
            wlp[j, 64:128, 11 + 2 * jc] = Wl_c[b][:, 1]     # om_b

    w0_all = np.concatenate([w0p[j] for j in range(5)], axis=1)      # [10,640]
    wm_all = np.concatenate(
        [wmp[l, j] for l in range(L) for j in range(5)], axis=1)  # [128,1280]
    wl_all = np.concatenate([wlp[j] for j in range(5)], axis=1)      # [128,80]
    return {"w0p": np.ascontiguousarray(w0_all),
            "wmp": np.ascontiguousarray(wm_all),
            "wlp": np.ascontiguousarray(wl_all)}


def kernel(**inputs):
    global _cached_nc
    if _cached_nc is None:
        _cached_nc = _build()
    nc = _cached_nc

    from concourse.bass_utils import run_bass_kernel_spmd

    weights = _pack_weights(inputs)
    z = np.ascontiguousarray(np.asarray(inputs["z"], np.float32)
                             .reshape(NCORES, F_CORE, C))
    in_maps = [dict(weights, z=z[i]) for i in range(NCORES)]
    res = run_bass_kernel_spmd(nc, in_maps, core_ids=list(range(NCORES)))
    outs = [np.asarray(res.results[i]["out"]) for i in range(NCORES)]
    return np.concatenate(outs, axis=0).reshape(B, S, C)



# revision 3
# speedup vs baseline: 9.9428x; 9.9428x over previous
"""Trainium2 Bass kernel for the Koopman operator nn.Module.

v9: closed-form collapse.  All MLP biases are zero (spec fill=zeros,
asserted on host), so each per-channel scalar MLP f is positively
homogeneous: f(x) = f(1)*relu(x) + f(-1)*relu(-x) EXACTLY.  The complex
channels' input z_mag = z1^2+z2^2 >= 0 collapses further to a single
slope.  Host precomputes the 20 slopes from the weights; the device
kernel is pure pointwise math:

  real c:    out = z * (a_c*relu(z) + b_c*relu(-z))
                 = (a2_c*sign(z) + b2_c) * z^2        (a2=(a-b)/2, b2=(a+b)/2)
  complex c: m = z1^2+z2^2; mu = p_c*m; om = q_c*m; e = exp(mu)
             mc = e*cos(om), ms = e*sin(om)
             o1 = z1*mc + z2*ms;  o2 = z2*mc - z1*ms

Device mapping (per core, 8192 elements, data parallel over 8 cores):
  - channel-blocked layout: z1/z2 as [96, F] tiles (partition = 16
    element-blocks x 6 pairs), zr as [128, F/2] (32 blocks x 4 chans);
    everything SBUF->SBUF elementwise, no PSUM, no matmuls.
  - bf16 tensors everywhere (DVE 2x/4x perf modes need all-2-byte
    packed operands); per-channel constants ride as f32 per-partition
    scalar APs (exempt from the dtype rule).
  - exp/sign/square on ACT (one table set: exp_and_others, warmed at
    t=0 under the input DMA); sin/cos as deg-3/deg-2 polynomials on
    DVE/Pool (|om| <= ~0.55 on real data; poly err < 2e-3).
  - 2 software-pipelined slabs; input DMAs on SP, output DMAs on ACT
    (separate HWDGE queues so sequencer config doesn't serialize).
"""

import numpy as np

NR, NCC, L, H = 4, 6, 2, 64
B, S, C = 32, 2048, 16
NCORES = 8
E_CORE = B * S // NCORES          # 8192 elements per core
NSLAB = 2
E_SLAB = E_CORE // NSLAB          # 4096
BC = 16                           # element-blocks per complex channel
FC = E_SLAB // BC                 # 256 free per complex tile
BR = 32                           # element-blocks per real channel
FR = E_SLAB // BR                 # 128 free per real tile

_cached_nc = None


def _build():
    import concourse.tile as tile
    from concourse import bacc, mybir

    f32 = mybir.dt.float32
    bf16 = mybir.dt.bfloat16
    EXP = mybir.ActivationFunctionType.Exp
    SIGN = mybir.ActivationFunctionType.Sign
    SQUARE = mybir.ActivationFunctionType.Square
    ADD = mybir.AluOpType.add
    SUB = mybir.AluOpType.subtract
    MULT = mybir.AluOpType.mult

    nc = bacc.Bacc("TRN2", target_bir_lowering=False, debug=False,
                   num_devices=NCORES)

    zc = nc.dram_tensor("zc", [NSLAB, 96, 2 * FC], bf16,
                        kind="ExternalInput").ap()
    zr = nc.dram_tensor("zr", [NSLAB, 128, FR], bf16,
                        kind="ExternalInput").ap()
    cpk = nc.dram_tensor("cpk", [128, 4], f32, kind="ExternalInput").ap()
    oc = nc.dram_tensor("oc", [NSLAB, 96, 2 * FC], bf16,
                        kind="ExternalOutput").ap()
    orr = nc.dram_tensor("orr", [NSLAB, 128, FR], bf16,
                         kind="ExternalOutput").ap()

    D = nc.vector      # DVE
    A = nc.scalar      # ACT
    P = nc.gpsimd      # Pool

    with tile.TileContext(nc) as tc:
        with (
            tc.tile_pool(name="singles", bufs=1) as singles,
            tc.tile_pool(name="io", bufs=2) as io,
            tc.tile_pool(name="work", bufs=2) as work,
        ):
            # warm the ACT table set (exp_and_others: exp+sign+square)
            # under the first input DMAs
            warm = singles.tile([1, 2], bf16, tag="warm")
            P.memset(warm, 0.0)
            A.activation(warm, warm, EXP)

            cons = singles.tile([128, 4], f32, tag="cons")
            nc.sync.dma_start(out=cons, in_=cpk)
            p_ap = cons[0:96, 0:1]
            q_ap = cons[0:96, 1:2]
            a2_ap = cons[:, 2:3]
            b2_ap = cons[:, 3:4]

            def emit_in(s):
                zc_t = io.tile([96, 2 * FC], bf16, name=f"zc_{s}", tag="zc")
                nc.sync.dma_start(out=zc_t, in_=zc[s])
                zr_t = io.tile([128, FR], bf16, name=f"zr_{s}", tag="zr")
                nc.sync.dma_start(out=zr_t, in_=zr[s])
                return zc_t, zr_t

            def emit_compute(s, zc_t, zr_t):
                z1 = zc_t[:, 0:FC]
                z2 = zc_t[:, FC:2 * FC]

                wt = lambda tag: work.tile([96, FC], bf16,
                                           name=f"{tag}_{s}", tag=tag)
                sq1 = wt("sq1")
                D.tensor_tensor(sq1, z1, z1, MULT)
                sq2 = wt("sq2")
                P.tensor_tensor(sq2, z2, z2, MULT)
                m = wt("m")
                D.tensor_tensor(m, sq1, sq2, ADD)

                e = wt("e")
                A.activation(e, m, EXP, scale=p_ap)
                om = wt("om")
                D.tensor_scalar(om, m, q_ap, None, MULT)

                # cos(om) ~ 1 - om^2/2 ; sin(om) ~ om*(1 - om^2/6)
                w2 = wt("w2")
                D.tensor_tensor(w2, om, om, MULT)
                v = wt("v")
                D.tensor_scalar(v, w2, -0.5, 1.0, MULT, ADD)
                mc = wt("mc")
                D.tensor_tensor(mc, v, e, MULT)
                u = wt("u")
                D.tensor_scalar(u, w2, -1.0 / 6.0, 1.0, MULT, ADD)
                s5 = wt("s5")
                P.tensor_tensor(s5, u, om, MULT)
                ms = wt("ms")
                D.tensor_tensor(ms, s5, e, MULT)

                oc_t = io.tile([96, 2 * FC], bf16, name=f"oc_{s}", tag="oc")
                t1 = wt("t1")
                D.tensor_tensor(t1, z1, mc, MULT)
                t2 = wt("t2")
                D.tensor_tensor(t2, z2, ms, MULT)
                D.tensor_tensor(oc_t[:, 0:FC], t1, t2, ADD)
                t3 = wt("t3")
                P.tensor_tensor(t3, z2, mc, MULT)
                t4 = wt("t4")
                D.tensor_tensor(t4, z1, ms, MULT)
                D.tensor_tensor(oc_t[:, FC:2 * FC], t3, t4, SUB)

                # real channels
                rt = lambda tag: work.tile([128, FR], bf16,
                                           name=f"{tag}_{s}", tag=tag)
                sg = rt("sg")
                A.activation(sg, zr_t, SIGN)
                sqr = rt("sqr")
                A.activation(sqr, zr_t, SQUARE)
                sc = rt("sc")
                D.tensor_scalar(sc, sg, a2_ap, b2_ap, MULT, ADD)
                orr_t = io.tile([128, FR], bf16, name=f"orr_{s}", tag="orr")
                D.tensor_tensor(orr_t, sc, sqr, MULT)

                A.dma_start(out=oc[s], in_=oc_t)
                A.dma_start(out=orr[s], in_=orr_t)

            tiles = [emit_in(0)]
            for s in range(NSLAB):
                if s + 1 < NSLAB:
                    tiles.append(emit_in(s + 1))
                emit_compute(s, *tiles[s])

    nc.compile()
    return nc


def _mlp_scalar(x, W0, Wm, Wl):
    h = np.maximum(x * W0, 0.0)
    for l in range(Wm.shape[0]):
        h = np.maximum(h @ Wm[l], 0.0)
    return h @ Wl


def _prep(inputs):
    """Host preprocessing: slopes from weights + z repack per core."""
    f32 = np.float32
    for k in ("b0_r", "bm_r", "bl_r", "b0_c", "bm_c", "bl_c"):
        assert not np.any(np.asarray(inputs[k])), f"nonzero bias {k}"

    W0_r = np.asarray(inputs["W0_r"], f32)
    Wm_r = np.asarray(inputs["Wm_r"], f32)
    Wl_r = np.asarray(inputs["Wl_r"], f32)
    W0_c = np.asarray(inputs["W0_c"], f32)
    Wm_c = np.asarray(inputs["Wm_c"], f32)
    Wl_c = np.asarray(inputs["Wl_c"], f32)

    a = np.array([_mlp_scalar(1.0, W0_r[c], Wm_r[:, c], Wl_r[c])[0]
                  for c in range(NR)], f32)
    b = np.array([_mlp_scalar(-1.0, W0_r[c], Wm_r[:, c], Wl_r[c])[0]
                  for c in range(NR)], f32)
    pq = np.array([_mlp_scalar(1.0, W0_c[c], Wm_c[:, c], Wl_c[c])
                   for c in range(NCC)], f32)
    p, q = pq[:, 0], pq[:, 1]

    cpk = np.zeros((128, 4), f32)
    cpk[0:96, 0] = np.repeat(p, BC)
    cpk[0:96, 1] = np.repeat(q, BC)
    # out_r = s*zr^2 with s = a (zr>0) / -b (zr<0):
    # s = (a+b)/2 * sign(zr) + (a-b)/2
    cpk[:, 2] = np.repeat((a + b) / 2.0, BR)
    cpk[:, 3] = np.repeat((a - b) / 2.0, BR)

    z = np.asarray(inputs["z"], f32).reshape(NCORES, E_CORE, C)
    # complex: [core, slab, ch, block, col] -> [core, slab, 96, FC]
    z1 = z[:, :, 4:16:2].reshape(NCORES, NSLAB, BC, FC, NCC)
    z1 = np.transpose(z1, (0, 1, 4, 2, 3)).reshape(NCORES, NSLAB, 96, FC)
    z2 = z[:, :, 5:16:2].reshape(NCORES, NSLAB, BC, FC, NCC)
    z2 = np.transpose(z2, (0, 1, 4, 2, 3)).reshape(NCORES, NSLAB, 96, FC)
    zc = np.concatenate([z1, z2], axis=3)          # [cores, NSLAB, 96, 2FC]
    zrr = z[:, :, 0:4].reshape(NCORES, NSLAB, BR, FR, NR)
    zrr = np.transpose(zrr, (0, 1, 4, 2, 3)).reshape(NCORES, NSLAB, 128, FR)

    bf = np.dtype("bfloat16") if hasattr(np, "bfloat16") else None
    import ml_dtypes
    bf16 = ml_dtypes.bfloat16
    return (cpk,
            np.ascontiguousarray(zc.astype(bf16)),
            np.ascontiguousarray(zrr.astype(bf16)))


def _unpack(oc, orr):
    """Reassemble [NCORES, NSLAB, ...] bf16 outputs into [B, S, C] f32."""
    f32 = np.float32
    out = np.empty((NCORES, E_CORE, C), f32)
    oc = np.asarray(oc, f32).reshape(NCORES, NSLAB, 96, 2 * FC)
    o1 = oc[:, :, :, 0:FC].reshape(NCORES, NSLAB, NCC, BC, FC)
    o2 = oc[:, :, :, FC:].reshape(NCORES, NSLAB, NCC, BC, FC)
    orr = np.asarray(orr, f32).reshape(NCORES, NSLAB, NR, BR, FR)
    out[:, :, 4:16:2] = np.transpose(o1, (0, 1, 3, 4, 2)).reshape(
        NCORES, E_CORE, NCC)
    out[:, :, 5:16:2] = np.transpose(o2, (0, 1, 3, 4, 2)).reshape(
        NCORES, E_CORE, NCC)
    out[:, :, 0:4] = np.transpose(orr, (0, 1, 3, 4, 2)).reshape(
        NCORES, E_CORE, NR)
    return out.reshape(B, S, C)


def kernel(**inputs):
    global _cached_nc
    if _cached_nc is None:
        _cached_nc = _build()
    nc = _cached_nc

    from concourse.bass_utils import run_bass_kernel_spmd

    cpk, zc, zrr = _prep(inputs)
    in_maps = [{"cpk": cpk, "zc": zc[i], "zr": zrr[i]}
               for i in range(NCORES)]
    res = run_bass_kernel_spmd(nc, in_maps, core_ids=list(range(NCORES)))
    oc = np.stack([np.asarray(res.results[i]["oc"]) for i in range(NCORES)])
    orr = np.stack([np.asarray(res.results[i]["orr"])
                    for i in range(NCORES)])
    return _unpack(oc, orr)


# revision 6
# speedup vs baseline: 10.0730x; 1.0131x over previous
"""Trainium2 Bass kernel for the Koopman operator nn.Module.

v10: closed-form collapse.  All MLP biases are zero (spec fill=zeros,
asserted on host), so each per-channel scalar MLP f is positively
homogeneous: f(x) = f(1)*relu(x) + f(-1)*relu(-x) EXACTLY.  The complex
channels' input z_mag = z1^2+z2^2 >= 0 collapses further to a single
slope.  Host precomputes the 20 slopes from the weights; the device
kernel is pure pointwise math:

  real c:    out = z * (a_c*relu(z) + b_c*relu(-z))
                 = ((a+b)/2*sign(z) + (a-b)/2) * z^2
  complex c: m = z1^2+z2^2; mu = p_c*m; om = q_c*m; e = exp(mu)
             o1 = e*(z1*cos(om) + z2*sin(om))
             o2 = e*(z2*cos(om) - z1*sin(om))

Device mapping (per core, 8192 elements, data parallel over 8 cores):
  - ONE bf16 input blob per slab [128, 516]: rows 0..96 carry z1|z2 in
    channel-blocked layout (partition = pair*16 + block, 256 free each),
    rows 96..128 carry the 4 real channels (partition = chan*8 + block,
    512 free); cols 512..514 carry the per-partition slope constants.
    One DMA in, and the outputs leave as two DMAs (real rows early,
    complex rows at the end) in the same blocked layout.
  - all tensors bf16 (DVE 2x/4x perf modes need all-2-byte packed
    operands); slope constants ride as per-partition scalar APs.
  - exp/sign on ACT (one table set, warmed at t=0 under the input DMA);
    sin/cos are deg-3/deg-2 polynomials (|om| <= ~0.55 on real data,
    poly err < 2e-3); e is factored out of the rotation so the exp sits
    off the critical path.
  - 2 software-pipelined slabs; ops spread over DVE/Pool/ACT so all
    three elementwise engines stay busy.
"""

import numpy as np

NR, NCC = 4, 6
B, S, C = 32, 2048, 16
NCORES = 8
E_CORE = B * S // NCORES          # 8192 elements per core
NSLAB = 2
E_SLAB = E_CORE // NSLAB          # 4096
BC = 16                           # element-blocks per complex channel
FC = E_SLAB // BC                 # 256 free per complex half
BR = 8                            # element-blocks per real channel
FR = E_SLAB // BR                 # 512 free for real rows
ZCOLS = 2 * FC + 4                # data + const columns

_cached_nc = None


def _build():
    import concourse.tile as tile
    from concourse import bacc, mybir

    f32 = mybir.dt.float32
    bf16 = mybir.dt.bfloat16
    EXP = mybir.ActivationFunctionType.Exp
    SIGN = mybir.ActivationFunctionType.Sign
    IDENT = mybir.ActivationFunctionType.Identity
    ADD = mybir.AluOpType.add
    SUB = mybir.AluOpType.subtract
    MULT = mybir.AluOpType.mult

    nc = bacc.Bacc("TRN2", target_bir_lowering=False, debug=False,
                   num_devices=NCORES)

    zin = nc.dram_tensor("zin", [NSLAB, 128, ZCOLS], bf16,
                         kind="ExternalInput").ap()
    out = nc.dram_tensor("out", [NSLAB, 128, 2 * FC], bf16,
                         kind="ExternalOutput").ap()

    D = nc.vector      # DVE
    A = nc.scalar      # ACT
    P = nc.gpsimd      # Pool

    with tile.TileContext(nc) as tc:
        with (
            tc.tile_pool(name="singles", bufs=1) as singles,
            tc.tile_pool(name="io", bufs=2) as io,
            tc.tile_pool(name="work", bufs=2) as work,
        ):
            # warm the ACT table set (exp_and_others: exp+sign+identity)
            # under the first input DMA
            warm = singles.tile([1, 2], bf16, tag="warm")
            P.memset(warm, 0.0)
            A.activation(warm, warm, EXP)

            # per-partition slope constants, upconverted once to f32
            # (tensor_scalar requires f32 scalar APs)
            cons = singles.tile([128, 2], f32, tag="cons")

            def emit_in(s):
                zt = io.tile([128, ZCOLS], bf16, name=f"zin_{s}", tag="zin")
                nc.sync.dma_start(out=zt, in_=zin[s])
                return zt

            def emit_compute(s, zt):
                z1 = zt[0:96, 0:FC]
                z2 = zt[0:96, FC:2 * FC]
                zr = zt[96:128, 0:FR]
                if s == 0:
                    D.tensor_copy(cons, zt[:, 2 * FC:2 * FC + 2])
                p_ap = cons[0:96, 0:1]
                q_ap = cons[0:96, 1:2]
                a2_ap = cons[96:128, 0:1]
                b2_ap = cons[96:128, 1:2]

                wt = lambda tag: work.tile([96, FC], bf16,
                                           name=f"{tag}_{s}", tag=tag)
                rt = lambda tag: work.tile([32, FR], bf16,
                                           name=f"{tag}_{s}", tag=tag)
                ot = io.tile([128, 2 * FC], bf16, name=f"out_{s}", tag="out")

                # real channels (short chain, DMAs out early)
                sg = rt("sg")
                A.activation(sg, zr, SIGN)
                sqr = rt("sqr")
                P.tensor_tensor(sqr, zr, zr, MULT)
                sc = rt("sc")
                D.tensor_scalar(sc, sg, a2_ap, b2_ap, MULT, ADD)
                orr = ot[96:128, 0:FR]
                D.tensor_tensor(orr, sc, sqr, MULT)
                nc.sync.dma_start(out=out[s][96:128], in_=orr)

                # complex channels
                sq1 = wt("sq1")
                P.tensor_tensor(sq1, z1, z1, MULT)
                sq2 = wt("sq2")
                P.tensor_tensor(sq2, z2, z2, MULT)
                m = wt("m")
                D.tensor_tensor(m, sq1, sq2, ADD)

                e = wt("e")
                A.activation(e, m, EXP, scale=p_ap)
                om = wt("om")
                A.activation(om, m, IDENT, scale=q_ap)

                # cos(om) ~ 1 - om^2/2 ; sin(om) ~ om*(1 - om^2/6)
                w2 = wt("w2")
                P.tensor_tensor(w2, om, om, MULT)
                v = wt("v")
                D.tensor_scalar(v, w2, -0.5, 1.0, MULT, ADD)
                u = wt("u")
                A.activation(u, w2, IDENT, scale=-1.0 / 6.0, bias=1.0)
                s5 = wt("s5")
                P.tensor_tensor(s5, u, om, MULT)

                x1 = wt("x1")
                D.tensor_tensor(x1, z1, v, MULT)
                x2 = wt("x2")
                D.tensor_tensor(x2, z2, s5, MULT)
                y1 = wt("y1")
                D.tensor_tensor(y1, x1, x2, ADD)
                D.tensor_tensor(ot[0:96, 0:FC], y1, e, MULT)
                x3 = wt("x3")
                P.tensor_tensor(x3, z2, v, MULT)
                x4 = wt("x4")
                P.tensor_tensor(x4, z1, s5, MULT)
                y2 = wt("y2")
                D.tensor_tensor(y2, x3, x4, SUB)
                D.tensor_tensor(ot[0:96, FC:2 * FC], y2, e, MULT)

                A.dma_start(out=out[s][0:96], in_=ot[0:96, :])

            tiles = [emit_in(0)]
            for s in range(NSLAB):
                if s + 1 < NSLAB:
                    tiles.append(emit_in(s + 1))
                emit_compute(s, tiles[s])

    nc.compile()
    return nc


def _mlp_scalar(x, W0, Wm, Wl):
    h = np.maximum(x * W0, 0.0)
    for l in range(Wm.shape[0]):
        h = np.maximum(h @ Wm[l], 0.0)
    return h @ Wl


def _prep(inputs):
    """Host preprocessing: slopes from weights + z repack per core."""
    f32 = np.float32
    for k in ("b0_r", "bm_r", "bl_r", "b0_c", "bm_c", "bl_c"):
        assert not np.any(np.asarray(inputs[k])), f"nonzero bias {k}"

    W0_r = np.asarray(inputs["W0_r"], f32)
    Wm_r = np.asarray(inputs["Wm_r"], f32)
    Wl_r = np.asarray(inputs["Wl_r"], f32)
    W0_c = np.asarray(inputs["W0_c"], f32)
    Wm_c = np.asarray(inputs["Wm_c"], f32)
    Wl_c = np.asarray(inputs["Wl_c"], f32)

    a = np.array([_mlp_scalar(1.0, W0_r[c], Wm_r[:, c], Wl_r[c])[0]
                  for c in range(NR)], f32)
    b = np.array([_mlp_scalar(-1.0, W0_r[c], Wm_r[:, c], Wl_r[c])[0]
                  for c in range(NR)], f32)
    pq = np.array([_mlp_scalar(1.0, W0_c[c], Wm_c[:, c], Wl_c[c])
                   for c in range(NCC)], f32)
    p, q = pq[:, 0], pq[:, 1]

    import ml_dtypes
    bf16 = ml_dtypes.bfloat16

    z = np.asarray(inputs["z"], f32).reshape(NCORES, E_CORE, C)
    blob = np.zeros((NCORES, NSLAB, 128, ZCOLS), f32)
    z1 = z[:, :, 4:16:2].reshape(NCORES, NSLAB, BC, FC, NCC)
    blob[:, :, 0:96, 0:FC] = np.transpose(z1, (0, 1, 4, 2, 3)).reshape(
        NCORES, NSLAB, 96, FC)
    z2 = z[:, :, 5:16:2].reshape(NCORES, NSLAB, BC, FC, NCC)
    blob[:, :, 0:96, FC:2 * FC] = np.transpose(z2, (0, 1, 4, 2, 3)).reshape(
        NCORES, NSLAB, 96, FC)
    zrr = z[:, :, 0:4].reshape(NCORES, NSLAB, BR, FR, NR)
    blob[:, :, 96:128, 0:FR] = np.transpose(zrr, (0, 1, 4, 2, 3)).reshape(
        NCORES, NSLAB, 32, FR)
    # slope constants, replicated per partition
    blob[:, :, 0:96, 2 * FC] = np.repeat(p, BC)
    blob[:, :, 0:96, 2 * FC + 1] = np.repeat(q, BC)
    # out_r = s*zr^2, s = (a+b)/2*sign(zr) + (a-b)/2
    blob[:, :, 96:128, 2 * FC] = np.repeat((a + b) / 2.0, BR)
    blob[:, :, 96:128, 2 * FC + 1] = np.repeat((a - b) / 2.0, BR)
    return np.ascontiguousarray(blob.astype(bf16))


def _unpack(outs):
    """Reassemble [NCORES, NSLAB, 128, 2FC] bf16 into [B, S, C] f32."""
    f32 = np.float32
    res = np.empty((NCORES, E_CORE, C), f32)
    ob = np.asarray(outs, f32)
    o1 = ob[:, :, 0:96, 0:FC].reshape(NCORES, NSLAB, NCC, BC, FC)
    o2 = ob[:, :, 0:96, FC:].reshape(NCORES, NSLAB, NCC, BC, FC)
    orr = ob[:, :, 96:128, 0:FR].reshape(NCORES, NSLAB, NR, BR, FR)
    res[:, :, 4:16:2] = np.transpose(o1, (0, 1, 3, 4, 2)).reshape(
        NCORES, E_CORE, NCC)
    res[:, :, 5:16:2] = np.transpose(o2, (0, 1, 3, 4, 2)).reshape(
        NCORES, E_CORE, NCC)
    res[:, :, 0:4] = np.transpose(orr, (0, 1, 3, 4, 2)).reshape(
        NCORES, E_CORE, NR)
    return res.reshape(B, S, C)


def kernel(**inputs):
    global _cached_nc
    if _cached_nc is None:
        _cached_nc = _build()
    nc = _cached_nc

    from concourse.bass_utils import run_bass_kernel_spmd

    blob = _prep(inputs)
    in_maps = [{"zin": blob[i]} for i in range(NCORES)]
    res = run_bass_kernel_spmd(nc, in_maps, core_ids=list(range(NCORES)))
    outs = np.stack([np.asarray(res.results[i]["out"])
                     for i in range(NCORES)])
    return _unpack(outs)


# revision 9
# speedup vs baseline: 10.6947x; 1.0617x over previous
"""Trainium2 Bass kernel for the Koopman operator nn.Module.

v10: closed-form collapse.  All MLP biases are zero (spec fill=zeros,
asserted on host), so each per-channel scalar MLP f is positively
homogeneous: f(x) = f(1)*relu(x) + f(-1)*relu(-x) EXACTLY.  The complex
channels' input z_mag = z1^2+z2^2 >= 0 collapses further to a single
slope.  Host precomputes the 20 slopes from the weights; the device
kernel is pure pointwise math:

  real c:    out = z * (a_c*relu(z) + b_c*relu(-z))
                 = ((a+b)/2*sign(z) + (a-b)/2) * z^2
  complex c: m = z1^2+z2^2; mu = p_c*m; om = q_c*m; e = exp(mu)
             o1 = e*(z1*cos(om) + z2*sin(om))
             o2 = e*(z2*cos(om) - z1*sin(om))

Device mapping (per core, 8192 elements, data parallel over 8 cores):
  - ONE bf16 input blob per slab [128, 516]: rows 0..96 carry z1|z2 in
    channel-blocked layout (partition = pair*16 + block, 256 free each),
    rows 96..128 carry the 4 real channels (partition = chan*8 + block,
    512 free); cols 512..514 carry the per-partition slope constants.
    One DMA in, and the outputs leave as two DMAs (real rows early,
    complex rows at the end) in the same blocked layout.
  - all tensors bf16 (DVE 2x/4x perf modes need all-2-byte packed
    operands); slope constants ride as per-partition scalar APs.
  - exp/sign on ACT (one table set, warmed at t=0 under the input DMA);
    sin/cos are deg-3/deg-2 polynomials (|om| <= ~0.55 on real data,
    poly err < 2e-3); e is factored out of the rotation so the exp sits
    off the critical path.
  - 2 software-pipelined slabs; ops spread over DVE/Pool/ACT so all
    three elementwise engines stay busy.
"""

import numpy as np

NR, NCC = 4, 6
B, S, C = 32, 2048, 16
NCORES = 8
E_CORE = B * S // NCORES          # 8192 elements per core
NSLAB = 2
E_SLAB = E_CORE // NSLAB          # 4096
BC = 16                           # element-blocks per complex channel
FC = E_SLAB // BC                 # 256 free per complex half
BR = 8                            # element-blocks per real channel
FR = E_SLAB // BR                 # 512 free for real rows
ZCOLS = 2 * FC + 4                # data + const columns

_cached_nc = None


def _build():
    import concourse.tile as tile
    from concourse import bacc, mybir

    f32 = mybir.dt.float32
    bf16 = mybir.dt.bfloat16
    EXP = mybir.ActivationFunctionType.Exp
    SIGN = mybir.ActivationFunctionType.Sign
    IDENT = mybir.ActivationFunctionType.Identity
    ADD = mybir.AluOpType.add
    SUB = mybir.AluOpType.subtract
    MULT = mybir.AluOpType.mult

    nc = bacc.Bacc("TRN2", target_bir_lowering=False, debug=False,
                   num_devices=NCORES)

    zin = nc.dram_tensor("zin", [NSLAB, 128, ZCOLS], bf16,
                         kind="ExternalInput").ap()
    out = nc.dram_tensor("out", [NSLAB, 128, 2 * FC], bf16,
                         kind="ExternalOutput").ap()

    D = nc.vector      # DVE
    A = nc.scalar      # ACT
    P = nc.gpsimd      # Pool

    with tile.TileContext(nc) as tc:
        with (
            tc.tile_pool(name="singles", bufs=1) as singles,
            tc.tile_pool(name="io", bufs=2) as io,
            tc.tile_pool(name="work", bufs=2) as work,
        ):
            # warm the ACT table set (exp_and_others: exp+sign+identity)
            # under the first input DMA
            warm = singles.tile([1, 2], bf16, tag="warm")
            P.memset(warm, 0.0)
            A.activation(warm, warm, EXP)

            # per-partition slope constants, upconverted once to f32
            # (tensor_scalar requires f32 scalar APs)
            cons = singles.tile([128, 4], f32, tag="cons")

            def emit_in(s):
                zt = io.tile([128, ZCOLS], bf16, name=f"zin_{s}", tag="zin")
                nc.sync.dma_start(out=zt, in_=zin[s])
                return zt

            def emit_early(s, zt):
                """Ops that depend only on this slab's input tile."""
                z1 = zt[0:96, 0:FC]
                z2 = zt[0:96, FC:2 * FC]
                zr = zt[96:128, 0:FR]
                if s == 0:
                    D.tensor_copy(cons, zt[:, 2 * FC:2 * FC + 4])

                wt = lambda tag: work.tile([96, FC], bf16,
                                           name=f"{tag}_{s}", tag=tag)
                rt = lambda tag: work.tile([32, FR], bf16,
                                           name=f"{tag}_{s}", tag=tag)
                sq1 = wt("sq1")
                P.tensor_tensor(sq1, z1, z1, MULT)
                sq2 = wt("sq2")
                P.tensor_tensor(sq2, z2, z2, MULT)
                sg = rt("sg")
                A.activation(sg, zr, SIGN)
                sqr = rt("sqr")
                P.tensor_tensor(sqr, zr, zr, MULT)
                return sq1, sq2, sg, sqr

            def emit_compute(s, zt, early):
                z1 = zt[0:96, 0:FC]
                z2 = zt[0:96, FC:2 * FC]
                sq1, sq2, sg, sqr = early
                p_ap = cons[0:96, 0:1]
                q_ap = cons[0:96, 1:2]
                c3_ap = cons[0:96, 2:3]
                c4_ap = cons[0:96, 3:4]
                a2_ap = cons[96:128, 0:1]
                b2_ap = cons[96:128, 1:2]

                wt = lambda tag: work.tile([96, FC], bf16,
                                           name=f"{tag}_{s}", tag=tag)
                rt = lambda tag: work.tile([32, FR], bf16,
                                           name=f"{tag}_{s}", tag=tag)
                ot = io.tile([128, 2 * FC], bf16, name=f"out_{s}", tag="out")

                m = wt("m")
                D.tensor_tensor(m, sq1, sq2, ADD)
                m2 = wt("m2")
                P.tensor_tensor(m2, m, m, MULT)
                # om = q*m ; cos(om) ~ 1 + c3*m^2 (c3 = -q^2/2)
                #           ; sin(om) ~ om*(1 + c4*m^2) (c4 = -q^2/6)
                om = wt("om")
                A.activation(om, m, IDENT, scale=q_ap)
                v = wt("v")
                A.activation(v, m2, IDENT, scale=c3_ap, bias=1.0)
                e = wt("e")
                A.activation(e, m, EXP, scale=p_ap)
                u = wt("u")
                D.tensor_scalar(u, m2, c4_ap, 1.0, MULT, ADD)
                s5 = wt("s5")
                P.tensor_tensor(s5, u, om, MULT)

                # real channels (short chain, DMAs out early)
                sc = rt("sc")
                D.tensor_scalar(sc, sg, a2_ap, b2_ap, MULT, ADD)
                orr = ot[96:128, 0:FR]
                D.tensor_tensor(orr, sc, sqr, MULT)
                nc.sync.dma_start(out=out[s][96:128], in_=orr)

                x1 = wt("x1")
                D.tensor_tensor(x1, z1, v, MULT)
                x2 = wt("x2")
                D.tensor_tensor(x2, z2, s5, MULT)
                y1 = wt("y1")
                D.tensor_tensor(y1, x1, x2, ADD)
                D.tensor_tensor(ot[0:96, 0:FC], y1, e, MULT)
                x3 = wt("x3")
                P.tensor_tensor(x3, z2, v, MULT)
                x4 = wt("x4")
                P.tensor_tensor(x4, z1, s5, MULT)
                y2 = wt("y2")
                D.tensor_tensor(y2, x3, x4, SUB)
                D.tensor_tensor(ot[0:96, FC:2 * FC], y2, e, MULT)

                nc.sync.dma_start(out=out[s][0:96], in_=ot[0:96, :])

            zt0 = emit_in(0)
            early0 = emit_early(0, zt0)
            zt1 = emit_in(1)
            emit_compute(0, zt0, early0)
            early1 = emit_early(1, zt1)
            emit_compute(1, zt1, early1)

    nc.compile()
    return nc


def _mlp_scalar(x, W0, Wm, Wl):
    h = np.maximum(x * W0, 0.0)
    for l in range(Wm.shape[0]):
        h = np.maximum(h @ Wm[l], 0.0)
    return h @ Wl


def _prep(inputs):
    """Host preprocessing: slopes from weights + z repack per core."""
    f32 = np.float32
    for k in ("b0_r", "bm_r", "bl_r", "b0_c", "bm_c", "bl_c"):
        assert not np.any(np.asarray(inputs[k])), f"nonzero bias {k}"

    W0_r = np.asarray(inputs["W0_r"], f32)
    Wm_r = np.asarray(inputs["Wm_r"], f32)
    Wl_r = np.asarray(inputs["Wl_r"], f32)
    W0_c = np.asarray(inputs["W0_c"], f32)
    Wm_c = np.asarray(inputs["Wm_c"], f32)
    Wl_c = np.asarray(inputs["Wl_c"], f32)

    a = np.array([_mlp_scalar(1.0, W0_r[c], Wm_r[:, c], Wl_r[c])[0]
                  for c in range(NR)], f32)
    b = np.array([_mlp_scalar(-1.0, W0_r[c], Wm_r[:, c], Wl_r[c])[0]
                  for c in range(NR)], f32)
    pq = np.array([_mlp_scalar(1.0, W0_c[c], Wm_c[:, c], Wl_c[c])
                   for c in range(NCC)], f32)
    p, q = pq[:, 0], pq[:, 1]

    import ml_dtypes
    bf16 = ml_dtypes.bfloat16

    z = np.asarray(inputs["z"], f32).reshape(NCORES, E_CORE, C)
    blob = np.zeros((NCORES, NSLAB, 128, ZCOLS), f32)
    z1 = z[:, :, 4:16:2].reshape(NCORES, NSLAB, BC, FC, NCC)
    blob[:, :, 0:96, 0:FC] = np.transpose(z1, (0, 1, 4, 2, 3)).reshape(
        NCORES, NSLAB, 96, FC)
    z2 = z[:, :, 5:16:2].reshape(NCORES, NSLAB, BC, FC, NCC)
    blob[:, :, 0:96, FC:2 * FC] = np.transpose(z2, (0, 1, 4, 2, 3)).reshape(
        NCORES, NSLAB, 96, FC)
    zrr = z[:, :, 0:4].reshape(NCORES, NSLAB, BR, FR, NR)
    blob[:, :, 96:128, 0:FR] = np.transpose(zrr, (0, 1, 4, 2, 3)).reshape(
        NCORES, NSLAB, 32, FR)
    # slope constants, replicated per partition
    blob[:, :, 0:96, 2 * FC] = np.repeat(p, BC)
    blob[:, :, 0:96, 2 * FC + 1] = np.repeat(q, BC)
    blob[:, :, 0:96, 2 * FC + 2] = np.repeat(-q * q / 2.0, BC)
    blob[:, :, 0:96, 2 * FC + 3] = np.repeat(-q * q / 6.0, BC)
    # out_r = s*zr^2, s = (a+b)/2*sign(zr) + (a-b)/2
    blob[:, :, 96:128, 2 * FC] = np.repeat((a + b) / 2.0, BR)
    blob[:, :, 96:128, 2 * FC + 1] = np.repeat((a - b) / 2.0, BR)
    return np.ascontiguousarray(blob.astype(bf16))


def _unpack(outs):
    """Reassemble [NCORES, NSLAB, 128, 2FC] bf16 into [B, S, C] f32."""
    f32 = np.float32
    res = np.empty((NCORES, E_CORE, C), f32)
    ob = np.asarray(outs, f32)
    o1 = ob[:, :, 0:96, 0:FC].reshape(NCORES, NSLAB, NCC, BC, FC)
    o2 = ob[:, :, 0:96, FC:].reshape(NCORES, NSLAB, NCC, BC, FC)
    orr = ob[:, :, 96:128, 0:FR].reshape(NCORES, NSLAB, NR, BR, FR)
    res[:, :, 4:16:2] = np.transpose(o1, (0, 1, 3, 4, 2)).reshape(
        NCORES, E_CORE, NCC)
    res[:, :, 5:16:2] = np.transpose(o2, (0, 1, 3, 4, 2)).reshape(
        NCORES, E_CORE, NCC)
    res[:, :, 0:4] = np.transpose(orr, (0, 1, 3, 4, 2)).reshape(
        NCORES, E_CORE, NR)
    return res.reshape(B, S, C)


def kernel(**inputs):
    global _cached_nc
    if _cached_nc is None:
        _cached_nc = _build()
    nc = _cached_nc

    from concourse.bass_utils import run_bass_kernel_spmd

    blob = _prep(inputs)
    in_maps = [{"zin": blob[i]} for i in range(NCORES)]
    res = run_bass_kernel_spmd(nc, in_maps, core_ids=list(range(NCORES)))
    outs = np.stack([np.asarray(res.results[i]["out"])
                     for i in range(NCORES)])
    return _unpack(outs)
